# revision 34
# baseline (speedup 1.0000x reference)
"""Trainium2 Bass kernel for DistributedAFNO2D (v2).

Problem: x(2,768,256,256) f32; per-block (8 blocks of 96 ch) spectral MLP:
  out = irfft2( softshrink( W2*relu(W1*rfft2(x) + b1) + b2 ) ) + x
Block-diagonal channel mixing with shared-per-(u,v) complex 96x96 weights.

Sharding: block k -> core k (8 cores). No collectives. Each core handles
(2, 96, 256, 256) with its own block weights.

v2 changes vs baseline:
  - Spectral bias fold: out = irfft2(S + Z) instead of irfft2(S) + x
    (Z = rfft2(x) is already computed; drops the 50MB f32 x reload).
  - Merged DMAs (one per logical tile group) + dispatch spread across
    sync/gpsimd queues (baseline serialized 1.28ms of DMA dispatch on sync).
  - Phase B processes u in pairs with re/im-split psum tiles (8 matmuls
    [96,96,258] per 2u) and 8-u DMA granularity; eltwise ops balanced
    across ACT/DVE/GpSimd.
  - Copies balanced across scalar/vector/gpsimd engines.

Dataflow per core, per batch b:
  Phase A (per channel c):  x[h,w] -> Z[u, c, v] (rfft2 via DFT matmuls)
  Phase B (per u-pair):     S = softshrink(W2 relu(W1 Z + b1) + b2)
  Phase C (per channel c):  out = irfft2(S + Z)
"""
import os
import sys
import numpy as np

sys.path.insert(0, "/opt/trn_rl_repo")

import ml_dtypes

BF16 = ml_dtypes.bfloat16

H = 256
W = 256
NV = W // 2 + 1  # 129
BLK = 96
NCORES = 8
B = 2
LAM = 0.01


def make_host_consts():
    """All packed constant matrices (numpy bf16) via probing np.fft."""
    I = np.eye(H, dtype=np.float64)
    F = np.fft.fft(I, axis=0, norm='ortho')       # F[u,h]; F@x = fft(x)
    Fi = np.fft.ifft(I, axis=0, norm='ortho')     # Fi[h,u]
    CHr = F.real.T.copy()                          # [h,u]
    CHi = F.imag.T.copy()
    EWr = F.real.T[:, :NV].copy()                  # [w,v]
    EWi = F.imag.T[:, :NV].copy()
    CHIr = Fi.real.T.copy()                        # [u,h]
    CHIi = Fi.imag.T.copy()
    Ir = np.eye(NV)
    Gc = np.fft.irfft(Ir, n=W, axis=-1, norm='ortho')        # [v,w]
    Gs = np.fft.irfft(1j * Ir, n=W, axis=-1, norm='ortho')   # [v,w]

    c = {}
    # CHpack [2][128, 258]: rows h (chunk), cols [CHr u=0..128 | CHi u=0..128]
    # (u=129..255 reconstructed from conjugate symmetry of the real-input FFT)
    c['chpack'] = np.stack([
        np.concatenate([CHr[j * 128:(j + 1) * 128, 0:129], CHi[j * 128:(j + 1) * 128, 0:129]], axis=1)
        for j in range(2)])
    # R1 [2][128, 258] = [EWr | EWi]; R2 = [-EWi | EWr] rows w chunk
    c['r1'] = np.stack([
        np.concatenate([EWr[j * 128:(j + 1) * 128], EWi[j * 128:(j + 1) * 128]], axis=1)
        for j in range(2)])
    c['r2'] = np.stack([
        np.concatenate([-EWi[j * 128:(j + 1) * 128], EWr[j * 128:(j + 1) * 128]], axis=1)
        for j in range(2)])
    c['r2n'] = -c['r2']
    # The S2 conj-symmetry trick stores the uc=1 block of Z in DESCENDING u
    # order (partition p holds u = 255-p). The whole pipeline is pointwise in
    # u until phase C's u-contraction, so only the chunk-1 inverse-DFT
    # constants need their rows reversed to match.
    # CHIpack [2][128, 512]: rows u chunk, cols [CHIr-h | CHIi-h]
    c['chipack'] = np.stack([
        np.concatenate([CHIr[0:128], CHIi[0:128]], axis=1),
        np.concatenate([CHIr[255:127:-1], CHIi[255:127:-1]], axis=1)])
    # NCHI [2][128, 256] = -CHIi rows u chunk
    c['nchi'] = np.stack([-CHIi[0:128], -CHIi[255:127:-1]])
    # NCHIpack [2][128, 512] = [-CHIi-h | CHIr-h] (merged Qr/Qi imag-term rhs)
    c['nchipack'] = np.stack([
        np.concatenate([-CHIi[0:128], CHIr[0:128]], axis=1),
        np.concatenate([-CHIi[255:127:-1], CHIr[255:127:-1]], axis=1)])
    # G tiles rows v=1..128
    c['gc'] = Gc[1:129]
    c['gs'] = Gs[1:129]
    return {k: v.astype(BF16) for k, v in c.items()}


def make_weight_consts(w1k, w2k):
    """Weight matrices for one block. w1k/w2k: (96, 96, 2) [i, o, ri]."""
    return {
        'w1r': w1k[..., 0].astype(BF16),
        'w1i': w1k[..., 1].astype(BF16),
        'w1in': (-w1k[..., 1]).astype(BF16),
        'w2r': w2k[..., 0].astype(BF16),
        'w2i': w2k[..., 1].astype(BF16),
        'w2in': (-w2k[..., 1]).astype(BF16),
    }


def build_nc():
    import concourse.bass as bass
    import concourse.tile as tile
    from concourse import bacc, mybir

    dt = mybir.dt
    nc = bacc.Bacc("TRN2", target_bir_lowering=False, debug=False)

    # I/O (x pre-split: h = hc*128 + p)
    xbf = nc.dram_tensor("xbf", [B, BLK, 2, 128, 256], dt.bfloat16, kind="ExternalInput").ap()
    chpack = nc.dram_tensor("chpack", [2, 128, 258], dt.bfloat16, kind="ExternalInput").ap()
    r1 = nc.dram_tensor("r1", [2, 128, 258], dt.bfloat16, kind="ExternalInput").ap()
    r2 = nc.dram_tensor("r2", [2, 128, 258], dt.bfloat16, kind="ExternalInput").ap()
    r2n = nc.dram_tensor("r2n", [2, 128, 258], dt.bfloat16, kind="ExternalInput").ap()
    chipack = nc.dram_tensor("chipack", [2, 128, 512], dt.bfloat16, kind="ExternalInput").ap()
    nchi = nc.dram_tensor("nchi", [2, 128, 256], dt.bfloat16, kind="ExternalInput").ap()
    nchipack = nc.dram_tensor("nchipack", [2, 128, 512], dt.bfloat16, kind="ExternalInput").ap()
    gc = nc.dram_tensor("gc", [128, 256], dt.bfloat16, kind="ExternalInput").ap()
    gs = nc.dram_tensor("gs", [128, 256], dt.bfloat16, kind="ExternalInput").ap()
    wts = {n: nc.dram_tensor(n, [96, 96], dt.bfloat16, kind="ExternalInput").ap()
           for n in ['w1r', 'w1i', 'w1in', 'w2r', 'w2i', 'w2in']}
    b1cols = nc.dram_tensor("b1cols", [96, 2], dt.float32, kind="ExternalInput").ap()
    # b2 clamp bounds broadcast across partitions: [128, 4, 96]
    # cols j: 0 = lam-b2r, 1 = -lam-b2r, 2 = lam-b2i, 3 = -lam-b2i (per channel)
    b2bc = nc.dram_tensor("b2bc", [128, 4, 96], dt.float32, kind="ExternalInput").ap()
    out = nc.dram_tensor("out", [B, BLK, 2, 128, 256], dt.float32, kind="ExternalOutput").ap()

    # DRAM scratch.
    # zbuf: Z spectrum, layout [b, p, uc, c, v2] with u = uc*128 + p
    zbuf = nc.dram_tensor("zbuf", [B, 128, 2, BLK, 258], dt.bfloat16).ap()
    # sbuf_d: S spectrum, layout [b, c, uc, p, v2]
    sbuf_d = nc.dram_tensor("sbufd", [B, BLK, 2, 128, 258], dt.bfloat16).ap()

    with tile.TileContext(nc) as tc:
        from contextlib import ExitStack
        with ExitStack() as ctx:
            consts = ctx.enter_context(tc.tile_pool(name="consts", bufs=1))
            pa_x = ctx.enter_context(tc.tile_pool(name="pa_x", bufs=3))
            pa_y = ctx.enter_context(tc.tile_pool(name="pa_y", bufs=3))
            pa_z = ctx.enter_context(tc.tile_pool(name="pa_z", bufs=3))
            pb_z = ctx.enter_context(tc.tile_pool(name="pb_z", bufs=3))
            pb_s = ctx.enter_context(tc.tile_pool(name="pb_s", bufs=3))
            pc_in = ctx.enter_context(tc.tile_pool(name="pc_in", bufs=3))
            pc_q = ctx.enter_context(tc.tile_pool(name="pc_q", bufs=3))
            pc_o = ctx.enter_context(tc.tile_pool(name="pc_o", bufs=3))
            # PSUM: 4 tags x 2 bufs x 1 bank = 8 banks
            psum = ctx.enter_context(tc.tile_pool(name="psum", bufs=1, space="PSUM"))

            # ---- Load constants ----
            def chunked_const(name, ap_, ncols):
                ts = []
                for j in range(2):
                    t = consts.tile([128, ncols], dt.bfloat16, tag=f"{name}{j}", name=f"{name}{j}")
                    nc.sync.dma_start(out=t, in_=ap_[j])
                    ts.append(t)
                return ts

            t_ch = chunked_const("t_ch", chpack, 258)
            t_r1 = chunked_const("t_r1", r1, 258)
            t_r2 = chunked_const("t_r2", r2, 258)
            t_r2n = chunked_const("t_r2n", r2n, 258)
            t_chi = chunked_const("t_chi", chipack, 512)
            t_nchi = chunked_const("t_nchi", nchi, 256)
            t_nchip = chunked_const("t_nchip", nchipack, 512)
            t_gc = consts.tile([128, 256], dt.bfloat16, tag="t_gc", name="t_gc")
            nc.sync.dma_start(out=t_gc, in_=gc)
            t_gs = consts.tile([128, 256], dt.bfloat16, tag="t_gs", name="t_gs")
            nc.sync.dma_start(out=t_gs, in_=gs)
            t_w = {}
            for n, ap_ in wts.items():
                t_w[n] = consts.tile([96, 96], dt.bfloat16, tag=f"t_{n}", name=f"t_{n}")
                nc.sync.dma_start(out=t_w[n], in_=ap_)

            t_b2bc = consts.tile([128, 4, 96], dt.float32, tag="t_b2bc", name="t_b2bc")
            nc.sync.dma_start(out=t_b2bc, in_=b2bc)
            t_b1 = consts.tile([96, 2], dt.float32, tag="t_b1", name="t_b1")
            nc.sync.dma_start(out=t_b1, in_=b1cols)

            for b in range(B):
                # ================= Phase A =================
                # x[h,w] --(DFT_h)--> Y[w, u] --(DFT_w)--> Z[u, v]
                # Software-pipelined: S2(c-1) is emitted after S1(c) so the
                # psum->sbuf y copies of iteration c-1 hide behind S1(c).
                def a_stage1(c):
                    xt = pa_x.tile([128, 2, 256], dt.bfloat16, tag="xt", name="xt")
                    nc.gpsimd.dma_start(out=xt, in_=xbf[b, c].transpose([1, 0, 2]))
                    # S1 with conj symmetry: Y[u] for u=0..128 only (N=258);
                    # u=129..255 = conj(Y[256-u]) via slice tricks in S2.
                    ys = []
                    for wc in range(2):
                        psy = psum.tile([128, 258], dt.float32, tag="psA", name="psy", bufs=3)
                        nc.tensor.matmul(psy, lhsT=xt[:, 0, wc * 128:(wc + 1) * 128],
                                         rhs=t_ch[0], start=True, stop=False)
                        nc.tensor.matmul(psy, lhsT=xt[:, 1, wc * 128:(wc + 1) * 128],
                                         rhs=t_ch[1], start=False, stop=True)
                        # y layout: [Yr u=0..128 | Yi u=0..128]
                        y = pa_y.tile([128, 258], dt.bfloat16, tag=f"y{wc}", name=f"y{wc}")
                        if wc == 0:
                            nc.scalar.copy(y, psy)
                        else:
                            nc.vector.tensor_scalar_add(y, psy, 0.0)
                        ys.append(y)
                    return ys

                def a_stage2(c, ys):
                    zt = pa_z.tile([128, 2, 258], dt.bfloat16, tag="zt", name="zt")
                    for uc in range(2):
                        psz = psum.tile([128, 258], dt.float32, tag="psB", name="psz", bufs=2)
                        if uc == 0:
                            # u = 0..127: Y[u] directly
                            sre, sim, tr2 = slice(0, 128), slice(129, 257), t_r2
                        else:
                            # partition p holds u = 255-p via conj(Y[s]), s = p+1
                            sre, sim, tr2 = slice(1, 129), slice(130, 258), t_r2n
                        nc.tensor.matmul(psz, lhsT=ys[0][:, sre], rhs=t_r1[0], start=True, stop=False)
                        nc.tensor.matmul(psz, lhsT=ys[0][:, sim], rhs=tr2[0], start=False, stop=False)
                        nc.tensor.matmul(psz, lhsT=ys[1][:, sre], rhs=t_r1[1], start=False, stop=False)
                        nc.tensor.matmul(psz, lhsT=ys[1][:, sim], rhs=tr2[1], start=False, stop=True)
                        if uc == 0:
                            nc.scalar.copy(zt[:, uc, :], psz)
                        else:
                            nc.vector.tensor_scalar_add(zt[:, uc, :], psz, 0.0)
                    nc.sync.dma_start(out=zbuf[b, :, :, c, :], in_=zt)

                prev_a = None
                for c in range(BLK):
                    ys = a_stage1(c)
                    if prev_a is not None:
                        a_stage2(*prev_a)
                    prev_a = (c, ys)
                a_stage2(*prev_a)

                # ================= Phase B =================
                # per u: o2[c, v] = W2 relu(W1 Z + b1)  (raw mix2; softshrink
                # + b2 deferred to phase C). 8-u DMA groups, u-pairs per
                # matmul set, software-pipelined: mix2(prev) after mix1(cur).
                b_ctx = {}

                def b_load(uc, p0):
                    zr8 = pb_z.tile([96, 8, 129], dt.bfloat16, tag="zr8", name="zr8")
                    nc.sync.dma_start(
                        out=zr8, in_=zbuf[b, p0:p0 + 8, uc, :, 0:129].transpose([1, 0, 2]))
                    zi8 = pb_z.tile([96, 8, 129], dt.bfloat16, tag="zi8", name="zi8")
                    nc.sync.dma_start(
                        out=zi8, in_=zbuf[b, p0:p0 + 8, uc, :, 129:258].transpose([1, 0, 2]))
                    st8 = pb_s.tile([96, 8, 258], dt.bfloat16, tag="st8", name="st8")
                    return zr8, zi8, st8

                def b_stage1(task):
                    uc, p0, j, zr8, zi8, st8 = task
                    js = slice(2 * j, 2 * j + 2)
                    zrs, zis = zr8[:, js, :], zi8[:, js, :]
                    ps1r = psum.tile([96, 2, 129], dt.float32, tag="psA", name="ps1r", bufs=3)
                    ps1i = psum.tile([96, 2, 129], dt.float32, tag="psA", name="ps1i", bufs=3)
                    nc.tensor.matmul(ps1r, lhsT=t_w['w1r'], rhs=zrs, start=True, stop=False)
                    nc.tensor.matmul(ps1i, lhsT=t_w['w1r'], rhs=zis, start=True, stop=False)
                    nc.tensor.matmul(ps1r, lhsT=t_w['w1in'], rhs=zis, start=False, stop=True)
                    nc.tensor.matmul(ps1i, lhsT=t_w['w1i'], rhs=zrs, start=False, stop=True)
                    o1r = pb_s.tile([96, 2, 129], dt.bfloat16, tag="o1r", name="o1r")
                    nc.scalar.activation(o1r, ps1r, mybir.ActivationFunctionType.Relu,
                                         bias=t_b1[:, 0:1])
                    o1i = pb_s.tile([96, 2, 129], dt.bfloat16, tag="o1i", name="o1i")
                    nc.scalar.activation(o1i, ps1i, mybir.ActivationFunctionType.Relu,
                                         bias=t_b1[:, 1:2])
                    return o1r, o1i

                def b_stage2(task, o1r, o1i):
                    uc, p0, j, zr8, zi8, st8 = task
                    js = slice(2 * j, 2 * j + 2)
                    ps2r = psum.tile([96, 2, 129], dt.float32, tag="psB", name="ps2r", bufs=2)
                    ps2i = psum.tile([96, 2, 129], dt.float32, tag="psC", name="ps2i", bufs=2)
                    nc.tensor.matmul(ps2r, lhsT=t_w['w2r'], rhs=o1r, start=True, stop=False)
                    nc.tensor.matmul(ps2i, lhsT=t_w['w2r'], rhs=o1i, start=True, stop=False)
                    nc.tensor.matmul(ps2r, lhsT=t_w['w2in'], rhs=o1i, start=False, stop=True)
                    nc.tensor.matmul(ps2i, lhsT=t_w['w2i'], rhs=o1r, start=False, stop=True)
                    nc.vector.tensor_scalar_add(st8[:, js, 0:129], ps2r, 0.0)
                    nc.vector.tensor_scalar_add(st8[:, js, 129:258], ps2i, 0.0)
                    if j == 3:
                        nc.sync.dma_start(out=sbuf_d[b, :, uc, p0:p0 + 8, :], in_=st8)

                prev_b = None
                for uc in range(2):
                    for p0 in range(0, 128, 8):
                        grp = b_load(uc, p0)
                        for j in range(4):
                            task = (uc, p0, j) + grp
                            o1 = b_stage1(task)
                            if prev_b is not None:
                                b_stage2(prev_b[0], *prev_b[1])
                            prev_b = (task, o1)
                b_stage2(prev_b[0], *prev_b[1])

                # ================= Phase C =================
                # T = softshrink_b2(o2) + Z; out[h, w] = irfft2(T)
                # Software-pipelined: pso/ot/store of c-1 after psab of c.
                def c_stage1(c):
                    o2t = pc_in.tile([128, 2, 258], dt.bfloat16, tag="o2t", name="o2t")
                    nc.gpsimd.dma_start(out=o2t, in_=sbuf_d[b, c].transpose([1, 0, 2]))
                    ztc = pc_in.tile([128, 2, 258], dt.bfloat16, tag="ztc", name="ztc")
                    nc.sync.dma_start(out=ztc, in_=zbuf[b, :, :, c, :])
                    # cl = clamp(o2, -lam-b2, lam-b2) per re/im half
                    cl2 = pc_in.tile([128, 2, 258], dt.bfloat16, tag="cl2", name="cl2")
                    nc.vector.tensor_scalar(cl2[:, :, 0:129], o2t[:, :, 0:129],
                                            t_b2bc[:, 0, c:c + 1], t_b2bc[:, 1, c:c + 1],
                                            mybir.AluOpType.min, mybir.AluOpType.max)
                    nc.gpsimd.tensor_scalar(cl2[:, :, 129:258], o2t[:, :, 129:258],
                                            t_b2bc[:, 2, c:c + 1], t_b2bc[:, 3, c:c + 1],
                                            mybir.AluOpType.min, mybir.AluOpType.max)
                    # T = (o2 - cl) + Z
                    tt1 = pc_in.tile([128, 2, 258], dt.bfloat16, tag="tt1", name="tt1")
                    nc.vector.tensor_tensor(tt1, o2t, cl2, mybir.AluOpType.subtract)
                    tt = pc_in.tile([128, 2, 258], dt.bfloat16, tag="tt", name="tt")
                    nc.vector.tensor_tensor(tt, tt1, ztc, mybir.AluOpType.add)

                    # [QrT | QiT] in one psum bank: Qr = tr.CHIr - ti.CHIi ;
                    # Qi = tr.CHIi + ti.CHIr (chi = [CHIr|CHIi], nchip =
                    # [-CHIi|CHIr]). The 8 tiny DC (v=0) matmuls interleave
                    # between the big ones so their full-width LDWEIGHTS hide
                    # behind long streams.
                    psab = psum.tile([128, 512], dt.float32, tag="psA", name="psab", bufs=3)
                    psq = psum.tile([128, 2], dt.float32, tag="psD", name="psq", bufs=1)

                    def dc_mm(k):
                        hc, t = divmod(k, 4)
                        lhs = [t_chi[0], t_nchi[0], t_chi[1], t_nchi[1]][t]
                        rhs = [tt[:, 0, 0:1], tt[:, 0, 129:130],
                               tt[:, 1, 0:1], tt[:, 1, 129:130]][t]
                        hs = slice(hc * 128, (hc + 1) * 128)
                        nc.tensor.matmul(psq[:, hc:hc + 1], lhsT=lhs[:, hs], rhs=rhs,
                                         start=(k == 0), stop=(k == 7), skip_group_check=True)

                    nc.tensor.matmul(psab, lhsT=tt[:, 0, 1:129], rhs=t_chi[0], start=True, stop=False)
                    dc_mm(0)
                    dc_mm(1)
                    nc.tensor.matmul(psab, lhsT=tt[:, 1, 1:129], rhs=t_chi[1], start=False, stop=False)
                    dc_mm(2)
                    dc_mm(3)
                    nc.tensor.matmul(psab, lhsT=tt[:, 0, 130:258], rhs=t_nchip[0], start=False, stop=False)
                    dc_mm(4)
                    dc_mm(5)
                    nc.tensor.matmul(psab, lhsT=tt[:, 1, 130:258], rhs=t_nchip[1], start=False, stop=True)
                    dc_mm(6)
                    dc_mm(7)

                    qr = pc_q.tile([128, 256], dt.bfloat16, tag="qr", name="qr")
                    nc.scalar.copy(qr, psab[:, 0:256])
                    qi = pc_q.tile([128, 256], dt.bfloat16, tag="qi", name="qi")
                    nc.scalar.copy(qi, psab[:, 256:512])
                    q0 = pc_q.tile([128, 2], dt.float32, tag="q0", name="q0")
                    nc.vector.tensor_scalar_mul(q0, psq, 1.0 / 16.0)
                    return qr, qi, q0

                def c_stage2(c, qr, qi, q0):
                    ot = pc_o.tile([128, 2, 256], dt.float32, tag="ot", name="ot")
                    psos = []
                    for hc in range(2):
                        pso = psum.tile([128, 256], dt.float32, tag="psC", name="pso", bufs=2)
                        nc.tensor.matmul(pso, lhsT=qr[:, hc * 128:(hc + 1) * 128], rhs=t_gc,
                                         start=True, stop=False)
                        nc.tensor.matmul(pso, lhsT=qi[:, hc * 128:(hc + 1) * 128], rhs=t_gs,
                                         start=False, stop=True)
                        psos.append(pso)
                    nc.scalar.activation(ot[:, 0, :], psos[0],
                                         mybir.ActivationFunctionType.Identity,
                                         bias=q0[:, 0:1])
                    nc.vector.tensor_scalar_add(ot[:, 1, :], psos[1], q0[:, 1:2])
                    nc.sync.dma_start(out=out[b, c].transpose([1, 0, 2]), in_=ot)

                prev_c = None
                for c in range(BLK):
                    r = c_stage1(c)
                    if prev_c is not None:
                        c_stage2(prev_c[0], *prev_c[1])
                    prev_c = (c, r)
                c_stage2(prev_c[0], *prev_c[1])
    nc.compile()
    return nc


_NC_CACHE = {}


def _get_nc():
    if 'nc' not in _NC_CACHE:
        _NC_CACHE['nc'] = build_nc()
    return _NC_CACHE['nc']


def make_in_maps(x, w1, b1, w2, b2):
    hc = make_host_consts()
    x = np.ascontiguousarray(x, dtype=np.float32)
    in_maps = []
    for k in range(NCORES):
        xk = np.ascontiguousarray(x[:, BLK * k:BLK * (k + 1)])
        wk = make_weight_consts(w1[k], w2[k])
        b2k = b2[k, :, 0, 0, :]
        b2cols = np.stack([LAM - b2k[:, 0], -LAM - b2k[:, 0],
                           LAM - b2k[:, 1], -LAM - b2k[:, 1]], axis=1).astype(np.float32)
        b2bc = np.ascontiguousarray(np.broadcast_to(b2cols.T[None, :, :], (128, 4, 96)))
        b1cols = np.ascontiguousarray(b1[k, :, 0, 0, :], dtype=np.float32)
        m = dict(
            b1cols=b1cols,
            b2bc=b2bc,
            xbf=xk.astype(BF16).reshape(B, BLK, 2, 128, 256),
            chpack=hc['chpack'], r1=hc['r1'], r2=hc['r2'], r2n=hc['r2n'],
            chipack=hc['chipack'], nchi=hc['nchi'], nchipack=hc['nchipack'],
            gc=hc['gc'], gs=hc['gs'],
            **wk,
        )
        in_maps.append(m)
    return in_maps


def assemble(res):
    outs = [res.results[k]['out'].reshape(B, BLK, H, W) for k in range(NCORES)]
    return np.concatenate(outs, axis=1)


def kernel(x, w1, b1, w2, b2):
    from concourse.bass_utils import run_bass_kernel_spmd
    nc = _get_nc()
    in_maps = make_in_maps(np.asarray(x), np.asarray(w1), np.asarray(b1),
                           np.asarray(w2), np.asarray(b2))
    res = run_bass_kernel_spmd(nc, in_maps, core_ids=list(range(NCORES)))
    return assemble(res)


# revision 35
# speedup vs baseline: 1.0787x; 1.0787x over previous
"""Trainium2 Bass kernel for DistributedAFNO2D (v2).

Problem: x(2,768,256,256) f32; per-block (8 blocks of 96 ch) spectral MLP:
  out = irfft2( softshrink( W2*relu(W1*rfft2(x) + b1) + b2 ) ) + x
Block-diagonal channel mixing with shared-per-(u,v) complex 96x96 weights.

Sharding: block k -> core k (8 cores). No collectives. Each core handles
(2, 96, 256, 256) with its own block weights.

v2 changes vs baseline:
  - Spectral bias fold: out = irfft2(S + Z) instead of irfft2(S) + x
    (Z = rfft2(x) is already computed; drops the 50MB f32 x reload).
  - Merged DMAs (one per logical tile group) + dispatch spread across
    sync/gpsimd queues (baseline serialized 1.28ms of DMA dispatch on sync).
  - Phase B processes u in pairs with re/im-split psum tiles (8 matmuls
    [96,96,258] per 2u) and 8-u DMA granularity; eltwise ops balanced
    across ACT/DVE/GpSimd.
  - Copies balanced across scalar/vector/gpsimd engines.

Dataflow per core, per batch b:
  Phase A (per channel c):  x[h,w] -> Z[u, c, v] (rfft2 via DFT matmuls)
  Phase B (per u-pair):     S = softshrink(W2 relu(W1 Z + b1) + b2)
  Phase C (per channel c):  out = irfft2(S + Z)
"""
import os
import sys
import numpy as np

sys.path.insert(0, "/opt/trn_rl_repo")

import ml_dtypes

BF16 = ml_dtypes.bfloat16

H = 256
W = 256
NV = W // 2 + 1  # 129
BLK = 96
NCORES = 8
B = 2
LAM = 0.01


def make_host_consts():
    """All packed constant matrices (numpy bf16) via probing np.fft."""
    I = np.eye(H, dtype=np.float64)
    F = np.fft.fft(I, axis=0, norm='ortho')       # F[u,h]; F@x = fft(x)
    Fi = np.fft.ifft(I, axis=0, norm='ortho')     # Fi[h,u]
    CHr = F.real.T.copy()                          # [h,u]
    CHi = F.imag.T.copy()
    EWr = F.real.T[:, :NV].copy()                  # [w,v]
    EWi = F.imag.T[:, :NV].copy()
    CHIr = Fi.real.T.copy()                        # [u,h]
    CHIi = Fi.imag.T.copy()
    Ir = np.eye(NV)
    Gc = np.fft.irfft(Ir, n=W, axis=-1, norm='ortho')        # [v,w]
    Gs = np.fft.irfft(1j * Ir, n=W, axis=-1, norm='ortho')   # [v,w]

    c = {}
    # CHpack [2][128, 258]: rows h (chunk), cols [CHr u=0..128 | CHi u=0..128]
    # (u=129..255 reconstructed from conjugate symmetry of the real-input FFT)
    c['chpack'] = np.stack([
        np.concatenate([CHr[j * 128:(j + 1) * 128, 0:129], CHi[j * 128:(j + 1) * 128, 0:129]], axis=1)
        for j in range(2)])
    # R1 [2][128, 258] = [EWr | EWi]; R2 = [-EWi | EWr] rows w chunk
    c['r1'] = np.stack([
        np.concatenate([EWr[j * 128:(j + 1) * 128], EWi[j * 128:(j + 1) * 128]], axis=1)
        for j in range(2)])
    c['r2'] = np.stack([
        np.concatenate([-EWi[j * 128:(j + 1) * 128], EWr[j * 128:(j + 1) * 128]], axis=1)
        for j in range(2)])
    c['r2n'] = -c['r2']
    # The S2 conj-symmetry trick stores the uc=1 block of Z in DESCENDING u
    # order (partition p holds u = 255-p). The whole pipeline is pointwise in
    # u until phase C's u-contraction, so only the chunk-1 inverse-DFT
    # constants need their rows reversed to match.
    # CHIpack [2][128, 512]: rows u chunk, cols [CHIr-h | CHIi-h]
    c['chipack'] = np.stack([
        np.concatenate([CHIr[0:128], CHIi[0:128]], axis=1),
        np.concatenate([CHIr[255:127:-1], CHIi[255:127:-1]], axis=1)])
    # NCHI [2][128, 256] = -CHIi rows u chunk
    c['nchi'] = np.stack([-CHIi[0:128], -CHIi[255:127:-1]])
    # NCHIpack [2][128, 512] = [-CHIi-h | CHIr-h] (merged Qr/Qi imag-term rhs)
    c['nchipack'] = np.stack([
        np.concatenate([-CHIi[0:128], CHIr[0:128]], axis=1),
        np.concatenate([-CHIi[255:127:-1], CHIr[255:127:-1]], axis=1)])
    # G tiles rows v=1..128
    c['gc'] = Gc[1:129]
    c['gs'] = Gs[1:129]
    return {k: v.astype(BF16) for k, v in c.items()}


def make_weight_consts(w1k, w2k):
    """Weight matrices for one block. w1k/w2k: (96, 96, 2) [i, o, ri]."""
    return {
        'w1r': w1k[..., 0].astype(BF16),
        'w1i': w1k[..., 1].astype(BF16),
        'w1in': (-w1k[..., 1]).astype(BF16),
        'w2r': w2k[..., 0].astype(BF16),
        'w2i': w2k[..., 1].astype(BF16),
        'w2in': (-w2k[..., 1]).astype(BF16),
    }


def build_nc():
    import concourse.bass as bass
    import concourse.tile as tile
    from concourse import bacc, mybir

    dt = mybir.dt
    nc = bacc.Bacc("TRN2", target_bir_lowering=False, debug=False)

    # I/O (x pre-split: h = hc*128 + p)
    xbf = nc.dram_tensor("xbf", [B, BLK, 2, 128, 256], dt.bfloat16, kind="ExternalInput").ap()
    chpack = nc.dram_tensor("chpack", [2, 128, 258], dt.bfloat16, kind="ExternalInput").ap()
    r1 = nc.dram_tensor("r1", [2, 128, 258], dt.bfloat16, kind="ExternalInput").ap()
    r2 = nc.dram_tensor("r2", [2, 128, 258], dt.bfloat16, kind="ExternalInput").ap()
    r2n = nc.dram_tensor("r2n", [2, 128, 258], dt.bfloat16, kind="ExternalInput").ap()
    chipack = nc.dram_tensor("chipack", [2, 128, 512], dt.bfloat16, kind="ExternalInput").ap()
    nchi = nc.dram_tensor("nchi", [2, 128, 256], dt.bfloat16, kind="ExternalInput").ap()
    nchipack = nc.dram_tensor("nchipack", [2, 128, 512], dt.bfloat16, kind="ExternalInput").ap()
    gc = nc.dram_tensor("gc", [128, 256], dt.bfloat16, kind="ExternalInput").ap()
    gs = nc.dram_tensor("gs", [128, 256], dt.bfloat16, kind="ExternalInput").ap()
    wts = {n: nc.dram_tensor(n, [96, 96], dt.bfloat16, kind="ExternalInput").ap()
           for n in ['w1r', 'w1i', 'w1in', 'w2r', 'w2i', 'w2in']}
    b1cols = nc.dram_tensor("b1cols", [96, 2], dt.float32, kind="ExternalInput").ap()
    # b2 clamp bounds broadcast across partitions: [128, 4, 96]
    # cols j: 0 = lam-b2r, 1 = -lam-b2r, 2 = lam-b2i, 3 = -lam-b2i (per channel)
    b2bc = nc.dram_tensor("b2bc", [128, 4, 96], dt.float32, kind="ExternalInput").ap()
    out = nc.dram_tensor("out", [B, BLK, 2, 128, 256], dt.float32, kind="ExternalOutput").ap()

    # DRAM scratch.
    # zbuf: Z spectrum, layout [b, p, uc, c, v2] with u = uc*128 + p
    zbuf = nc.dram_tensor("zbuf", [B, 128, 2, BLK, 258], dt.bfloat16).ap()
    # sbuf_d: S spectrum, layout [b, c, uc, p, v2]
    sbuf_d = nc.dram_tensor("sbufd", [B, BLK, 2, 128, 258], dt.bfloat16).ap()

    with tile.TileContext(nc) as tc:
        from contextlib import ExitStack
        with ExitStack() as ctx:
            consts = ctx.enter_context(tc.tile_pool(name="consts", bufs=1))
            pa_x = ctx.enter_context(tc.tile_pool(name="pa_x", bufs=4))
            pa_y = ctx.enter_context(tc.tile_pool(name="pa_y", bufs=3))
            pa_z = ctx.enter_context(tc.tile_pool(name="pa_z", bufs=3))
            pb_z = ctx.enter_context(tc.tile_pool(name="pb_z", bufs=4))
            pb_s = ctx.enter_context(tc.tile_pool(name="pb_s", bufs=3))
            pc_in = ctx.enter_context(tc.tile_pool(name="pc_in", bufs=4))
            pc_q = ctx.enter_context(tc.tile_pool(name="pc_q", bufs=3))
            pc_o = ctx.enter_context(tc.tile_pool(name="pc_o", bufs=3))
            # PSUM: 4 tags x 2 bufs x 1 bank = 8 banks
            psum = ctx.enter_context(tc.tile_pool(name="psum", bufs=1, space="PSUM"))

            # ---- Load constants ----
            def chunked_const(name, ap_, ncols):
                ts = []
                for j in range(2):
                    t = consts.tile([128, ncols], dt.bfloat16, tag=f"{name}{j}", name=f"{name}{j}")
                    nc.sync.dma_start(out=t, in_=ap_[j])
                    ts.append(t)
                return ts

            t_ch = chunked_const("t_ch", chpack, 258)
            t_r1 = chunked_const("t_r1", r1, 258)
            t_r2 = chunked_const("t_r2", r2, 258)
            t_r2n = chunked_const("t_r2n", r2n, 258)
            t_chi = chunked_const("t_chi", chipack, 512)
            t_nchi = chunked_const("t_nchi", nchi, 256)
            t_nchip = chunked_const("t_nchip", nchipack, 512)
            t_gc = consts.tile([128, 256], dt.bfloat16, tag="t_gc", name="t_gc")
            nc.sync.dma_start(out=t_gc, in_=gc)
            t_gs = consts.tile([128, 256], dt.bfloat16, tag="t_gs", name="t_gs")
            nc.sync.dma_start(out=t_gs, in_=gs)
            t_w = {}
            for n, ap_ in wts.items():
                t_w[n] = consts.tile([96, 96], dt.bfloat16, tag=f"t_{n}", name=f"t_{n}")
                nc.sync.dma_start(out=t_w[n], in_=ap_)

            t_b2bc = consts.tile([128, 4, 96], dt.float32, tag="t_b2bc", name="t_b2bc")
            nc.sync.dma_start(out=t_b2bc, in_=b2bc)
            t_b1 = consts.tile([96, 2], dt.float32, tag="t_b1", name="t_b1")
            nc.sync.dma_start(out=t_b1, in_=b1cols)

            for b in range(B):
                # ================= Phase A =================
                # x[h,w] --(DFT_h)--> Y[w, u] --(DFT_w)--> Z[u, v]
                for c in range(BLK):
                    xt = pa_x.tile([128, 2, 256], dt.bfloat16, tag="xt", name="xt")
                    nc.gpsimd.dma_start(out=xt, in_=xbf[b, c].transpose([1, 0, 2]))

                    # S1 with conj symmetry: Y[u] computed for u=0..128 only
                    # (N=258 matmuls); u=129..255 = conj(Y[256-u]) via the
                    # shifted uc=1 slices in S2 + host-reversed C constants.
                    ys = []
                    for wc in range(2):
                        psy = psum.tile([128, 258], dt.float32, tag="psA", name="psy", bufs=3)
                        nc.tensor.matmul(psy, lhsT=xt[:, 0, wc * 128:(wc + 1) * 128],
                                         rhs=t_ch[0], start=True, stop=False)
                        nc.tensor.matmul(psy, lhsT=xt[:, 1, wc * 128:(wc + 1) * 128],
                                         rhs=t_ch[1], start=False, stop=True)
                        # y layout: [Yr u=0..128 | Yi u=0..128]
                        y = pa_y.tile([128, 258], dt.bfloat16, tag=f"y{wc}", name=f"y{wc}")
                        if wc == 0:
                            nc.scalar.copy(y, psy)
                        else:
                            nc.vector.tensor_scalar_add(y, psy, 0.0)
                        ys.append(y)

                    zt = pa_z.tile([128, 2, 258], dt.bfloat16, tag="zt", name="zt")
                    for uc in range(2):
                        psz = psum.tile([128, 258], dt.float32, tag="psB", name="psz", bufs=2)
                        if uc == 0:
                            # u = 0..127: Y[u] directly
                            sre, sim, tr2 = slice(0, 128), slice(129, 257), t_r2
                        else:
                            # partition p holds u = 255-p via conj(Y[s]), s = p+1
                            sre, sim, tr2 = slice(1, 129), slice(130, 258), t_r2n
                        nc.tensor.matmul(psz, lhsT=ys[0][:, sre], rhs=t_r1[0], start=True, stop=False)
                        nc.tensor.matmul(psz, lhsT=ys[0][:, sim], rhs=tr2[0], start=False, stop=False)
                        nc.tensor.matmul(psz, lhsT=ys[1][:, sre], rhs=t_r1[1], start=False, stop=False)
                        nc.tensor.matmul(psz, lhsT=ys[1][:, sim], rhs=tr2[1], start=False, stop=True)
                        if uc == 0:
                            nc.scalar.copy(zt[:, uc, :], psz)
                        else:
                            nc.vector.tensor_scalar_add(zt[:, uc, :], psz, 0.0)
                    nc.sync.dma_start(out=zbuf[b, :, :, c, :], in_=zt)

                # ================= Phase B =================
                # per u: o2[c, v] = W2 relu(W1 Z + b1) raw mix2
                # (softshrink + b2 deferred to phase C via clamp bounds)
                # processed as 8-u DMA groups, pairs of u per matmul set
                for uc in range(2):
                    for p0 in range(0, 128, 8):
                        zr8 = pb_z.tile([96, 8, 129], dt.bfloat16, tag="zr8", name="zr8")
                        nc.sync.dma_start(
                            out=zr8, in_=zbuf[b, p0:p0 + 8, uc, :, 0:129].transpose([1, 0, 2]))
                        zi8 = pb_z.tile([96, 8, 129], dt.bfloat16, tag="zi8", name="zi8")
                        nc.sync.dma_start(
                            out=zi8, in_=zbuf[b, p0:p0 + 8, uc, :, 129:258].transpose([1, 0, 2]))
                        st8 = pb_s.tile([96, 8, 258], dt.bfloat16, tag="st8", name="st8")

                        for j in range(4):
                            js = slice(2 * j, 2 * j + 2)
                            zrs = zr8[:, js, :]
                            zis = zi8[:, js, :]
                            # mix1: o1r = W1r Zr - W1i Zi ; o1i = W1i Zr + W1r Zi
                            ps1r = psum.tile([96, 2, 129], dt.float32, tag="psA", name="ps1r", bufs=3)
                            ps1i = psum.tile([96, 2, 129], dt.float32, tag="psA", name="ps1i", bufs=3)
                            nc.tensor.matmul(ps1r, lhsT=t_w['w1r'], rhs=zrs, start=True, stop=False)
                            nc.tensor.matmul(ps1i, lhsT=t_w['w1r'], rhs=zis, start=True, stop=False)
                            nc.tensor.matmul(ps1r, lhsT=t_w['w1in'], rhs=zis, start=False, stop=True)
                            nc.tensor.matmul(ps1i, lhsT=t_w['w1i'], rhs=zrs, start=False, stop=True)

                            o1r = pb_s.tile([96, 2, 129], dt.bfloat16, tag="o1r", name="o1r")
                            nc.scalar.activation(o1r, ps1r, mybir.ActivationFunctionType.Relu,
                                                 bias=t_b1[:, 0:1])
                            o1i = pb_s.tile([96, 2, 129], dt.bfloat16, tag="o1i", name="o1i")
                            nc.scalar.activation(o1i, ps1i, mybir.ActivationFunctionType.Relu,
                                                 bias=t_b1[:, 1:2])

                            ps2r = psum.tile([96, 2, 129], dt.float32, tag="psB", name="ps2r", bufs=2)
                            ps2i = psum.tile([96, 2, 129], dt.float32, tag="psC", name="ps2i", bufs=2)
                            nc.tensor.matmul(ps2r, lhsT=t_w['w2r'], rhs=o1r, start=True, stop=False)
                            nc.tensor.matmul(ps2i, lhsT=t_w['w2r'], rhs=o1i, start=True, stop=False)
                            nc.tensor.matmul(ps2r, lhsT=t_w['w2in'], rhs=o1i, start=False, stop=True)
                            nc.tensor.matmul(ps2i, lhsT=t_w['w2i'], rhs=o1r, start=False, stop=True)

                            nc.vector.tensor_scalar_add(st8[:, js, 0:129], ps2r, 0.0)
                            nc.vector.tensor_scalar_add(st8[:, js, 129:258], ps2i, 0.0)

                        nc.sync.dma_start(out=sbuf_d[b, :, uc, p0:p0 + 8, :], in_=st8)

                # ================= Phase C =================
                # T = softshrink_b2(o2) + Z; out[h, w] = irfft2(T)
                for c in range(BLK):
                    o2t = pc_in.tile([128, 2, 258], dt.bfloat16, tag="o2t", name="o2t")
                    nc.gpsimd.dma_start(out=o2t, in_=sbuf_d[b, c].transpose([1, 0, 2]))
                    ztc = pc_in.tile([128, 2, 258], dt.bfloat16, tag="ztc", name="ztc")
                    nc.sync.dma_start(out=ztc, in_=zbuf[b, :, :, c, :])
                    # cl = clamp(o2, -lam-b2, lam-b2) per re/im half
                    cl2 = pc_in.tile([128, 2, 258], dt.bfloat16, tag="cl2", name="cl2")
                    nc.vector.tensor_scalar(cl2[:, :, 0:129], o2t[:, :, 0:129],
                                            t_b2bc[:, 0, c:c + 1], t_b2bc[:, 1, c:c + 1],
                                            mybir.AluOpType.min, mybir.AluOpType.max)
                    nc.gpsimd.tensor_scalar(cl2[:, :, 129:258], o2t[:, :, 129:258],
                                            t_b2bc[:, 2, c:c + 1], t_b2bc[:, 3, c:c + 1],
                                            mybir.AluOpType.min, mybir.AluOpType.max)
                    # T = (o2 - cl) + Z
                    tt1 = pc_in.tile([128, 2, 258], dt.bfloat16, tag="tt1", name="tt1")
                    nc.vector.tensor_tensor(tt1, o2t, cl2, mybir.AluOpType.subtract)
                    tt = pc_in.tile([128, 2, 258], dt.bfloat16, tag="tt", name="tt")
                    nc.vector.tensor_tensor(tt, tt1, ztc, mybir.AluOpType.add)

                    # [QrT | QiT] in one psum bank: Qr = tr.CHIr - ti.CHIi ;
                    # Qi = tr.CHIi + ti.CHIr (chi = [CHIr|CHIi], nchip = [-CHIi|CHIr])
                    # The 8 tiny DC (v=0) matmuls are interleaved between the big
                    # ones so their full-width LDWEIGHTS hide behind long streams.
                    psab = psum.tile([128, 512], dt.float32, tag="psA", name="psab", bufs=3)
                    psq = psum.tile([128, 2], dt.float32, tag="psD", name="psq", bufs=1)

                    def dc_mm(k):
                        hc, t = divmod(k, 4)
                        lhs = [t_chi[0], t_nchi[0], t_chi[1], t_nchi[1]][t]
                        rhs = [tt[:, 0, 0:1], tt[:, 0, 129:130],
                               tt[:, 1, 0:1], tt[:, 1, 129:130]][t]
                        hs = slice(hc * 128, (hc + 1) * 128)
                        nc.tensor.matmul(psq[:, hc:hc + 1], lhsT=lhs[:, hs], rhs=rhs,
                                         start=(k == 0), stop=(k == 7), skip_group_check=True)

                    nc.tensor.matmul(psab, lhsT=tt[:, 0, 1:129], rhs=t_chi[0], start=True, stop=False)
                    dc_mm(0)
                    nc.tensor.matmul(psab, lhsT=tt[:, 1, 1:129], rhs=t_chi[1], start=False, stop=False)
                    dc_mm(1)
                    nc.tensor.matmul(psab, lhsT=tt[:, 0, 130:258], rhs=t_nchip[0], start=False, stop=False)
                    dc_mm(2)
                    nc.tensor.matmul(psab, lhsT=tt[:, 1, 130:258], rhs=t_nchip[1], start=False, stop=True)
                    dc_mm(3)

                    qr = pc_q.tile([128, 256], dt.bfloat16, tag="qr", name="qr")
                    nc.scalar.copy(qr, psab[:, 0:256])
                    qi = pc_q.tile([128, 256], dt.bfloat16, tag="qi", name="qi")
                    nc.scalar.copy(qi, psab[:, 256:512])

                    ot = pc_o.tile([128, 2, 256], dt.float32, tag="ot", name="ot")
                    psos = []
                    for hc in range(2):
                        pso = psum.tile([128, 256], dt.float32, tag="psC", name="pso", bufs=2)
                        nc.tensor.matmul(pso, lhsT=qr[:, hc * 128:(hc + 1) * 128], rhs=t_gc,
                                         start=True, stop=False)
                        dc_mm(4 + 2 * hc)
                        nc.tensor.matmul(pso, lhsT=qi[:, hc * 128:(hc + 1) * 128], rhs=t_gs,
                                         start=False, stop=True)
                        dc_mm(5 + 2 * hc)
                        psos.append(pso)
                    q0 = pc_q.tile([128, 2], dt.float32, tag="q0", name="q0")
                    nc.vector.tensor_scalar_mul(q0, psq, 1.0 / 16.0)
                    nc.scalar.activation(ot[:, 0, :], psos[0],
                                         mybir.ActivationFunctionType.Identity,
                                         bias=q0[:, 0:1])
                    nc.vector.tensor_scalar_add(ot[:, 1, :], psos[1], q0[:, 1:2])
                    nc.sync.dma_start(out=out[b, c].transpose([1, 0, 2]), in_=ot)
    nc.compile()
    return nc


_NC_CACHE = {}


def _get_nc():
    if 'nc' not in _NC_CACHE:
        _NC_CACHE['nc'] = build_nc()
    return _NC_CACHE['nc']


def make_in_maps(x, w1, b1, w2, b2):
    hc = make_host_consts()
    x = np.ascontiguousarray(x, dtype=np.float32)
    in_maps = []
    for k in range(NCORES):
        xk = np.ascontiguousarray(x[:, BLK * k:BLK * (k + 1)])
        wk = make_weight_consts(w1[k], w2[k])
        b2k = b2[k, :, 0, 0, :]
        b2cols = np.stack([LAM - b2k[:, 0], -LAM - b2k[:, 0],
                           LAM - b2k[:, 1], -LAM - b2k[:, 1]], axis=1).astype(np.float32)
        b2bc = np.ascontiguousarray(np.broadcast_to(b2cols.T[None, :, :], (128, 4, 96)))
        b1cols = np.ascontiguousarray(b1[k, :, 0, 0, :], dtype=np.float32)
        m = dict(
            b1cols=b1cols,
            b2bc=b2bc,
            xbf=xk.astype(BF16).reshape(B, BLK, 2, 128, 256),
            chpack=hc['chpack'], r1=hc['r1'], r2=hc['r2'], r2n=hc['r2n'],
            chipack=hc['chipack'], nchi=hc['nchi'], nchipack=hc['nchipack'],
            gc=hc['gc'], gs=hc['gs'],
            **wk,
        )
        in_maps.append(m)
    return in_maps


def assemble(res):
    outs = [res.results[k]['out'].reshape(B, BLK, H, W) for k in range(NCORES)]
    return np.concatenate(outs, axis=1)


def kernel(x, w1, b1, w2, b2):
    from concourse.bass_utils import run_bass_kernel_spmd
    nc = _get_nc()
    in_maps = make_in_maps(np.asarray(x), np.asarray(w1), np.asarray(b1),
                           np.asarray(w2), np.asarray(b2))
    res = run_bass_kernel_spmd(nc, in_maps, core_ids=list(range(NCORES)))
    return assemble(res)


# revision 36
# speedup vs baseline: 1.1103x; 1.0294x over previous
"""Trainium2 Bass kernel for DistributedAFNO2D (v2).

Problem: x(2,768,256,256) f32; per-block (8 blocks of 96 ch) spectral MLP:
  out = irfft2( softshrink( W2*relu(W1*rfft2(x) + b1) + b2 ) ) + x
Block-diagonal channel mixing with shared-per-(u,v) complex 96x96 weights.

Sharding: block k -> core k (8 cores). No collectives. Each core handles
(2, 96, 256, 256) with its own block weights.

v2 changes vs baseline:
  - Spectral bias fold: out = irfft2(S + Z) instead of irfft2(S) + x
    (Z = rfft2(x) is already computed; drops the 50MB f32 x reload).
  - Merged DMAs (one per logical tile group) + dispatch spread across
    sync/gpsimd queues (baseline serialized 1.28ms of DMA dispatch on sync).
  - Phase B processes u in pairs with re/im-split psum tiles (8 matmuls
    [96,96,258] per 2u) and 8-u DMA granularity; eltwise ops balanced
    across ACT/DVE/GpSimd.
  - Copies balanced across scalar/vector/gpsimd engines.

Dataflow per core, per batch b:
  Phase A (per channel c):  x[h,w] -> Z[u, c, v] (rfft2 via DFT matmuls)
  Phase B (per u-pair):     S = softshrink(W2 relu(W1 Z + b1) + b2)
  Phase C (per channel c):  out = irfft2(S + Z)
"""
import os
import sys
import numpy as np

sys.path.insert(0, "/opt/trn_rl_repo")

import ml_dtypes

BF16 = ml_dtypes.bfloat16

H = 256
W = 256
NV = W // 2 + 1  # 129
BLK = 96
NCORES = 8
B = 2
LAM = 0.01


def make_host_consts():
    """All packed constant matrices (numpy bf16) via probing np.fft."""
    I = np.eye(H, dtype=np.float64)
    F = np.fft.fft(I, axis=0, norm='ortho')       # F[u,h]; F@x = fft(x)
    Fi = np.fft.ifft(I, axis=0, norm='ortho')     # Fi[h,u]
    CHr = F.real.T.copy()                          # [h,u]
    CHi = F.imag.T.copy()
    EWr = F.real.T[:, :NV].copy()                  # [w,v]
    EWi = F.imag.T[:, :NV].copy()
    CHIr = Fi.real.T.copy()                        # [u,h]
    CHIi = Fi.imag.T.copy()
    Ir = np.eye(NV)
    Gc = np.fft.irfft(Ir, n=W, axis=-1, norm='ortho')        # [v,w]
    Gs = np.fft.irfft(1j * Ir, n=W, axis=-1, norm='ortho')   # [v,w]

    c = {}
    # CHpack [2][128, 258]: rows h (chunk), cols [CHr u=0..128 | CHi u=0..128]
    # (u=129..255 reconstructed from conjugate symmetry of the real-input FFT)
    c['chpack'] = np.stack([
        np.concatenate([CHr[j * 128:(j + 1) * 128, 0:129], CHi[j * 128:(j + 1) * 128, 0:129]], axis=1)
        for j in range(2)])
    # R1 [2][128, 258] = [EWr | EWi]; R2 = [-EWi | EWr] rows w chunk
    c['r1'] = np.stack([
        np.concatenate([EWr[j * 128:(j + 1) * 128], EWi[j * 128:(j + 1) * 128]], axis=1)
        for j in range(2)])
    c['r2'] = np.stack([
        np.concatenate([-EWi[j * 128:(j + 1) * 128], EWr[j * 128:(j + 1) * 128]], axis=1)
        for j in range(2)])
    c['r2n'] = -c['r2']
    # The S2 conj-symmetry trick stores the uc=1 block of Z in DESCENDING u
    # order (partition p holds u = 255-p). The whole pipeline is pointwise in
    # u until phase C's u-contraction, so only the chunk-1 inverse-DFT
    # constants need their rows reversed to match.
    # CHIpack [2][128, 512]: rows u chunk, cols [CHIr-h | CHIi-h]
    c['chipack'] = np.stack([
        np.concatenate([CHIr[0:128], CHIi[0:128]], axis=1),
        np.concatenate([CHIr[255:127:-1], CHIi[255:127:-1]], axis=1)])
    # NCHI [2][128, 256] = -CHIi rows u chunk
    c['nchi'] = np.stack([-CHIi[0:128], -CHIi[255:127:-1]])
    # NCHIpack [2][128, 512] = [-CHIi-h | CHIr-h] (merged Qr/Qi imag-term rhs)
    c['nchipack'] = np.stack([
        np.concatenate([-CHIi[0:128], CHIr[0:128]], axis=1),
        np.concatenate([-CHIi[255:127:-1], CHIr[255:127:-1]], axis=1)])
    # G tiles rows v=1..128
    c['gc'] = Gc[1:129]
    c['gs'] = Gs[1:129]
    return {k: v.astype(BF16) for k, v in c.items()}


def make_weight_consts(w1k, w2k):
    """Weight matrices for one block. w1k/w2k: (96, 96, 2) [i, o, ri]."""
    return {
        'w1r': w1k[..., 0].astype(BF16),
        'w1i': w1k[..., 1].astype(BF16),
        'w1in': (-w1k[..., 1]).astype(BF16),
        'w2r': w2k[..., 0].astype(BF16),
        'w2i': w2k[..., 1].astype(BF16),
        'w2in': (-w2k[..., 1]).astype(BF16),
    }


def build_nc():
    import concourse.bass as bass
    import concourse.tile as tile
    from concourse import bacc, mybir

    dt = mybir.dt
    nc = bacc.Bacc("TRN2", target_bir_lowering=False, debug=False)

    # I/O (x pre-split: h = hc*128 + p)
    xbf = nc.dram_tensor("xbf", [B, BLK, 2, 128, 256], dt.bfloat16, kind="ExternalInput").ap()
    chpack = nc.dram_tensor("chpack", [2, 128, 258], dt.bfloat16, kind="ExternalInput").ap()
    r1 = nc.dram_tensor("r1", [2, 128, 258], dt.bfloat16, kind="ExternalInput").ap()
    r2 = nc.dram_tensor("r2", [2, 128, 258], dt.bfloat16, kind="ExternalInput").ap()
    r2n = nc.dram_tensor("r2n", [2, 128, 258], dt.bfloat16, kind="ExternalInput").ap()
    chipack = nc.dram_tensor("chipack", [2, 128, 512], dt.bfloat16, kind="ExternalInput").ap()
    nchi = nc.dram_tensor("nchi", [2, 128, 256], dt.bfloat16, kind="ExternalInput").ap()
    nchipack = nc.dram_tensor("nchipack", [2, 128, 512], dt.bfloat16, kind="ExternalInput").ap()
    gc = nc.dram_tensor("gc", [128, 256], dt.bfloat16, kind="ExternalInput").ap()
    gs = nc.dram_tensor("gs", [128, 256], dt.bfloat16, kind="ExternalInput").ap()
    wts = {n: nc.dram_tensor(n, [96, 96], dt.bfloat16, kind="ExternalInput").ap()
           for n in ['w1r', 'w1i', 'w1in', 'w2r', 'w2i', 'w2in']}
    b1cols = nc.dram_tensor("b1cols", [96, 2], dt.float32, kind="ExternalInput").ap()
    # b2 clamp bounds broadcast across partitions: [128, 4, 96]
    # cols j: 0 = lam-b2r, 1 = -lam-b2r, 2 = lam-b2i, 3 = -lam-b2i (per channel)
    b2bc = nc.dram_tensor("b2bc", [128, 4, 96], dt.float32, kind="ExternalInput").ap()
    out = nc.dram_tensor("out", [B, BLK, 2, 128, 256], dt.float32, kind="ExternalOutput").ap()

    # DRAM scratch.
    # zbuf: Z spectrum, layout [b, p, uc, c, v2] with u = uc*128 + p
    zbuf = nc.dram_tensor("zbuf", [B, 128, 2, BLK, 258], dt.bfloat16).ap()
    # sbuf_d: S spectrum, layout [b, c, uc, p, v2]
    sbuf_d = nc.dram_tensor("sbufd", [B, BLK, 2, 128, 258], dt.bfloat16).ap()

    with tile.TileContext(nc) as tc:
        from contextlib import ExitStack
        with ExitStack() as ctx:
            consts = ctx.enter_context(tc.tile_pool(name="consts", bufs=1))
            pa_x = ctx.enter_context(tc.tile_pool(name="pa_x", bufs=6))
            pa_y = ctx.enter_context(tc.tile_pool(name="pa_y", bufs=5))
            pa_z = ctx.enter_context(tc.tile_pool(name="pa_z", bufs=5))
            pb_z = ctx.enter_context(tc.tile_pool(name="pb_z", bufs=6))
            pb_s = ctx.enter_context(tc.tile_pool(name="pb_s", bufs=5))
            pc_in = ctx.enter_context(tc.tile_pool(name="pc_in", bufs=6))
            pc_q = ctx.enter_context(tc.tile_pool(name="pc_q", bufs=5))
            pc_o = ctx.enter_context(tc.tile_pool(name="pc_o", bufs=5))
            # PSUM: 4 tags x 2 bufs x 1 bank = 8 banks
            psum = ctx.enter_context(tc.tile_pool(name="psum", bufs=1, space="PSUM"))

            # ---- Load constants ----
            def chunked_const(name, ap_, ncols):
                ts = []
                for j in range(2):
                    t = consts.tile([128, ncols], dt.bfloat16, tag=f"{name}{j}", name=f"{name}{j}")
                    nc.sync.dma_start(out=t, in_=ap_[j])
                    ts.append(t)
                return ts

            t_ch = chunked_const("t_ch", chpack, 258)
            t_r1 = chunked_const("t_r1", r1, 258)
            t_r2 = chunked_const("t_r2", r2, 258)
            t_r2n = chunked_const("t_r2n", r2n, 258)
            t_chi = chunked_const("t_chi", chipack, 512)
            t_nchi = chunked_const("t_nchi", nchi, 256)
            t_nchip = chunked_const("t_nchip", nchipack, 512)
            t_gc = consts.tile([128, 256], dt.bfloat16, tag="t_gc", name="t_gc")
            nc.sync.dma_start(out=t_gc, in_=gc)
            t_gs = consts.tile([128, 256], dt.bfloat16, tag="t_gs", name="t_gs")
            nc.sync.dma_start(out=t_gs, in_=gs)
            t_w = {}
            for n, ap_ in wts.items():
                t_w[n] = consts.tile([96, 96], dt.bfloat16, tag=f"t_{n}", name=f"t_{n}")
                nc.sync.dma_start(out=t_w[n], in_=ap_)

            t_b2bc = consts.tile([128, 4, 96], dt.float32, tag="t_b2bc", name="t_b2bc")
            nc.sync.dma_start(out=t_b2bc, in_=b2bc)
            t_b1 = consts.tile([96, 2], dt.float32, tag="t_b1", name="t_b1")
            nc.sync.dma_start(out=t_b1, in_=b1cols)

            for b in range(B):
                # ================= Phase A =================
                # x[h,w] --(DFT_h)--> Y[w, u] --(DFT_w)--> Z[u, v]
                for c in range(BLK):
                    xt = pa_x.tile([128, 2, 256], dt.bfloat16, tag="xt", name="xt")
                    nc.gpsimd.dma_start(out=xt, in_=xbf[b, c].transpose([1, 0, 2]))

                    # S1 with conj symmetry: Y[u] computed for u=0..128 only
                    # (N=258 matmuls); u=129..255 = conj(Y[256-u]) via the
                    # shifted uc=1 slices in S2 + host-reversed C constants.
                    ys = []
                    for wc in range(2):
                        psy = psum.tile([128, 258], dt.float32, tag="psA", name="psy", bufs=3)
                        nc.tensor.matmul(psy, lhsT=xt[:, 0, wc * 128:(wc + 1) * 128],
                                         rhs=t_ch[0], start=True, stop=False)
                        nc.tensor.matmul(psy, lhsT=xt[:, 1, wc * 128:(wc + 1) * 128],
                                         rhs=t_ch[1], start=False, stop=True)
                        # y layout: [Yr u=0..128 | Yi u=0..128]
                        y = pa_y.tile([128, 258], dt.bfloat16, tag=f"y{wc}", name=f"y{wc}")
                        if wc == 0:
                            nc.scalar.copy(y, psy)
                        else:
                            nc.vector.tensor_scalar_add(y, psy, 0.0)
                        ys.append(y)

                    zt = pa_z.tile([128, 2, 258], dt.bfloat16, tag="zt", name="zt")
                    for uc in range(2):
                        psz = psum.tile([128, 258], dt.float32, tag="psB", name="psz", bufs=2)
                        if uc == 0:
                            # u = 0..127: Y[u] directly
                            sre, sim, tr2 = slice(0, 128), slice(129, 257), t_r2
                        else:
                            # partition p holds u = 255-p via conj(Y[s]), s = p+1
                            sre, sim, tr2 = slice(1, 129), slice(130, 258), t_r2n
                        nc.tensor.matmul(psz, lhsT=ys[0][:, sre], rhs=t_r1[0], start=True, stop=False)
                        nc.tensor.matmul(psz, lhsT=ys[0][:, sim], rhs=tr2[0], start=False, stop=False)
                        nc.tensor.matmul(psz, lhsT=ys[1][:, sre], rhs=t_r1[1], start=False, stop=False)
                        nc.tensor.matmul(psz, lhsT=ys[1][:, sim], rhs=tr2[1], start=False, stop=True)
                        if uc == 0:
                            nc.scalar.copy(zt[:, uc, :], psz)
                        else:
                            nc.vector.tensor_scalar_add(zt[:, uc, :], psz, 0.0)
                    nc.sync.dma_start(out=zbuf[b, :, :, c, :], in_=zt)

                # ================= Phase B =================
                # per u: o2[c, v] = W2 relu(W1 Z + b1) raw mix2
                # (softshrink + b2 deferred to phase C via clamp bounds)
                # processed as 8-u DMA groups, pairs of u per matmul set
                for uc in range(2):
                    for p0 in range(0, 128, 8):
                        zr8 = pb_z.tile([96, 8, 129], dt.bfloat16, tag="zr8", name="zr8")
                        nc.sync.dma_start(
                            out=zr8, in_=zbuf[b, p0:p0 + 8, uc, :, 0:129].transpose([1, 0, 2]))
                        zi8 = pb_z.tile([96, 8, 129], dt.bfloat16, tag="zi8", name="zi8")
                        nc.sync.dma_start(
                            out=zi8, in_=zbuf[b, p0:p0 + 8, uc, :, 129:258].transpose([1, 0, 2]))
                        st8 = pb_s.tile([96, 8, 258], dt.bfloat16, tag="st8", name="st8")

                        for j in range(4):
                            js = slice(2 * j, 2 * j + 2)
                            zrs = zr8[:, js, :]
                            zis = zi8[:, js, :]
                            # mix1: o1r = W1r Zr - W1i Zi ; o1i = W1i Zr + W1r Zi
                            ps1r = psum.tile([96, 2, 129], dt.float32, tag="psA", name="ps1r", bufs=3)
                            ps1i = psum.tile([96, 2, 129], dt.float32, tag="psA", name="ps1i", bufs=3)
                            nc.tensor.matmul(ps1r, lhsT=t_w['w1r'], rhs=zrs, start=True, stop=False)
                            nc.tensor.matmul(ps1i, lhsT=t_w['w1r'], rhs=zis, start=True, stop=False)
                            nc.tensor.matmul(ps1r, lhsT=t_w['w1in'], rhs=zis, start=False, stop=True)
                            nc.tensor.matmul(ps1i, lhsT=t_w['w1i'], rhs=zrs, start=False, stop=True)

                            o1r = pb_s.tile([96, 2, 129], dt.bfloat16, tag="o1r", name="o1r")
                            nc.scalar.activation(o1r, ps1r, mybir.ActivationFunctionType.Relu,
                                                 bias=t_b1[:, 0:1])
                            o1i = pb_s.tile([96, 2, 129], dt.bfloat16, tag="o1i", name="o1i")
                            nc.scalar.activation(o1i, ps1i, mybir.ActivationFunctionType.Relu,
                                                 bias=t_b1[:, 1:2])

                            ps2r = psum.tile([96, 2, 129], dt.float32, tag="psB", name="ps2r", bufs=2)
                            ps2i = psum.tile([96, 2, 129], dt.float32, tag="psC", name="ps2i", bufs=2)
                            nc.tensor.matmul(ps2r, lhsT=t_w['w2r'], rhs=o1r, start=True, stop=False)
                            nc.tensor.matmul(ps2i, lhsT=t_w['w2r'], rhs=o1i, start=True, stop=False)
                            nc.tensor.matmul(ps2r, lhsT=t_w['w2in'], rhs=o1i, start=False, stop=True)
                            nc.tensor.matmul(ps2i, lhsT=t_w['w2i'], rhs=o1r, start=False, stop=True)

                            nc.vector.tensor_scalar_add(st8[:, js, 0:129], ps2r, 0.0)
                            nc.vector.tensor_scalar_add(st8[:, js, 129:258], ps2i, 0.0)

                        nc.sync.dma_start(out=sbuf_d[b, :, uc, p0:p0 + 8, :], in_=st8)

                # ================= Phase C =================
                # T = softshrink_b2(o2) + Z; out[h, w] = irfft2(T)
                for c in range(BLK):
                    o2t = pc_in.tile([128, 2, 258], dt.bfloat16, tag="o2t", name="o2t")
                    nc.gpsimd.dma_start(out=o2t, in_=sbuf_d[b, c].transpose([1, 0, 2]))
                    ztc = pc_in.tile([128, 2, 258], dt.bfloat16, tag="ztc", name="ztc")
                    nc.sync.dma_start(out=ztc, in_=zbuf[b, :, :, c, :])
                    # cl = clamp(o2, -lam-b2, lam-b2) per re/im half
                    cl2 = pc_in.tile([128, 2, 258], dt.bfloat16, tag="cl2", name="cl2")
                    nc.vector.tensor_scalar(cl2[:, :, 0:129], o2t[:, :, 0:129],
                                            t_b2bc[:, 0, c:c + 1], t_b2bc[:, 1, c:c + 1],
                                            mybir.AluOpType.min, mybir.AluOpType.max)
                    nc.gpsimd.tensor_scalar(cl2[:, :, 129:258], o2t[:, :, 129:258],
                                            t_b2bc[:, 2, c:c + 1], t_b2bc[:, 3, c:c + 1],
                                            mybir.AluOpType.min, mybir.AluOpType.max)
                    # T = (o2 - cl) + Z
                    tt1 = pc_in.tile([128, 2, 258], dt.bfloat16, tag="tt1", name="tt1")
                    nc.vector.tensor_tensor(tt1, o2t, cl2, mybir.AluOpType.subtract)
                    tt = pc_in.tile([128, 2, 258], dt.bfloat16, tag="tt", name="tt")
                    nc.vector.tensor_tensor(tt, tt1, ztc, mybir.AluOpType.add)

                    # [QrT | QiT] in one psum bank: Qr = tr.CHIr - ti.CHIi ;
                    # Qi = tr.CHIi + ti.CHIr (chi = [CHIr|CHIi], nchip = [-CHIi|CHIr])
                    # The 8 tiny DC (v=0) matmuls are interleaved between the big
                    # ones so their full-width LDWEIGHTS hide behind long streams.
                    psab = psum.tile([128, 512], dt.float32, tag="psA", name="psab", bufs=3)
                    psq = psum.tile([128, 2], dt.float32, tag="psD", name="psq", bufs=1)

                    def dc_mm(k):
                        hc, t = divmod(k, 4)
                        lhs = [t_chi[0], t_nchi[0], t_chi[1], t_nchi[1]][t]
                        rhs = [tt[:, 0, 0:1], tt[:, 0, 129:130],
                               tt[:, 1, 0:1], tt[:, 1, 129:130]][t]
                        hs = slice(hc * 128, (hc + 1) * 128)
                        nc.tensor.matmul(psq[:, hc:hc + 1], lhsT=lhs[:, hs], rhs=rhs,
                                         start=(k == 0), stop=(k == 7), skip_group_check=True)

                    nc.tensor.matmul(psab, lhsT=tt[:, 0, 1:129], rhs=t_chi[0], start=True, stop=False)
                    dc_mm(0)
                    nc.tensor.matmul(psab, lhsT=tt[:, 1, 1:129], rhs=t_chi[1], start=False, stop=False)
                    dc_mm(1)
                    nc.tensor.matmul(psab, lhsT=tt[:, 0, 130:258], rhs=t_nchip[0], start=False, stop=False)
                    dc_mm(2)
                    nc.tensor.matmul(psab, lhsT=tt[:, 1, 130:258], rhs=t_nchip[1], start=False, stop=True)
                    dc_mm(3)

                    qr = pc_q.tile([128, 256], dt.bfloat16, tag="qr", name="qr")
                    nc.scalar.copy(qr, psab[:, 0:256])
                    qi = pc_q.tile([128, 256], dt.bfloat16, tag="qi", name="qi")
                    nc.scalar.copy(qi, psab[:, 256:512])

                    ot = pc_o.tile([128, 2, 256], dt.float32, tag="ot", name="ot")
                    psos = []
                    for hc in range(2):
                        pso = psum.tile([128, 256], dt.float32, tag="psC", name="pso", bufs=2)
                        nc.tensor.matmul(pso, lhsT=qr[:, hc * 128:(hc + 1) * 128], rhs=t_gc,
                                         start=True, stop=False)
                        dc_mm(4 + 2 * hc)
                        nc.tensor.matmul(pso, lhsT=qi[:, hc * 128:(hc + 1) * 128], rhs=t_gs,
                                         start=False, stop=True)
                        dc_mm(5 + 2 * hc)
                        psos.append(pso)
                    q0 = pc_q.tile([128, 2], dt.float32, tag="q0", name="q0")
                    nc.vector.tensor_scalar_mul(q0, psq, 1.0 / 16.0)
                    nc.scalar.activation(ot[:, 0, :], psos[0],
                                         mybir.ActivationFunctionType.Identity,
                                         bias=q0[:, 0:1])
                    nc.vector.tensor_scalar_add(ot[:, 1, :], psos[1], q0[:, 1:2])
                    nc.sync.dma_start(out=out[b, c].transpose([1, 0, 2]), in_=ot)
    nc.compile()
    return nc


_NC_CACHE = {}


def _get_nc():
    if 'nc' not in _NC_CACHE:
        _NC_CACHE['nc'] = build_nc()
    return _NC_CACHE['nc']


def make_in_maps(x, w1, b1, w2, b2):
    hc = make_host_consts()
    x = np.ascontiguousarray(x, dtype=np.float32)
    in_maps = []
    for k in range(NCORES):
        xk = np.ascontiguousarray(x[:, BLK * k:BLK * (k + 1)])
        wk = make_weight_consts(w1[k], w2[k])
        b2k = b2[k, :, 0, 0, :]
        b2cols = np.stack([LAM - b2k[:, 0], -LAM - b2k[:, 0],
                           LAM - b2k[:, 1], -LAM - b2k[:, 1]], axis=1).astype(np.float32)
        b2bc = np.ascontiguousarray(np.broadcast_to(b2cols.T[None, :, :], (128, 4, 96)))
        b1cols = np.ascontiguousarray(b1[k, :, 0, 0, :], dtype=np.float32)
        m = dict(
            b1cols=b1cols,
            b2bc=b2bc,
            xbf=xk.astype(BF16).reshape(B, BLK, 2, 128, 256),
            chpack=hc['chpack'], r1=hc['r1'], r2=hc['r2'], r2n=hc['r2n'],
            chipack=hc['chipack'], nchi=hc['nchi'], nchipack=hc['nchipack'],
            gc=hc['gc'], gs=hc['gs'],
            **wk,
        )
        in_maps.append(m)
    return in_maps


def assemble(res):
    outs = [res.results[k]['out'].reshape(B, BLK, H, W) for k in range(NCORES)]
    return np.concatenate(outs, axis=1)


def kernel(x, w1, b1, w2, b2):
    from concourse.bass_utils import run_bass_kernel_spmd
    nc = _get_nc()
    in_maps = make_in_maps(np.asarray(x), np.asarray(w1), np.asarray(b1),
                           np.asarray(w2), np.asarray(b2))
    res = run_bass_kernel_spmd(nc, in_maps, core_ids=list(range(NCORES)))
    return assemble(res)


# revision 37
# speedup vs baseline: 1.2180x; 1.0970x over previous
"""Trainium2 Bass kernel for DistributedAFNO2D (v2).

Problem: x(2,768,256,256) f32; per-block (8 blocks of 96 ch) spectral MLP:
  out = irfft2( softshrink( W2*relu(W1*rfft2(x) + b1) + b2 ) ) + x
Block-diagonal channel mixing with shared-per-(u,v) complex 96x96 weights.

Sharding: block k -> core k (8 cores). No collectives. Each core handles
(2, 96, 256, 256) with its own block weights.

v2 changes vs baseline:
  - Spectral bias fold: out = irfft2(S + Z) instead of irfft2(S) + x
    (Z = rfft2(x) is already computed; drops the 50MB f32 x reload).
  - Merged DMAs (one per logical tile group) + dispatch spread across
    sync/gpsimd queues (baseline serialized 1.28ms of DMA dispatch on sync).
  - Phase B processes u in pairs with re/im-split psum tiles (8 matmuls
    [96,96,258] per 2u) and 8-u DMA granularity; eltwise ops balanced
    across ACT/DVE/GpSimd.
  - Copies balanced across scalar/vector/gpsimd engines.

Dataflow per core, per batch b:
  Phase A (per channel c):  x[h,w] -> Z[u, c, v] (rfft2 via DFT matmuls)
  Phase B (per u-pair):     S = softshrink(W2 relu(W1 Z + b1) + b2)
  Phase C (per channel c):  out = irfft2(S + Z)
"""
import os
import sys
import numpy as np

sys.path.insert(0, "/opt/trn_rl_repo")

import ml_dtypes

BF16 = ml_dtypes.bfloat16

H = 256
W = 256
NV = W // 2 + 1  # 129
BLK = 96
NCORES = 8
B = 2
LAM = 0.01


def make_host_consts():
    """All packed constant matrices (numpy bf16) via probing np.fft."""
    I = np.eye(H, dtype=np.float64)
    F = np.fft.fft(I, axis=0, norm='ortho')       # F[u,h]; F@x = fft(x)
    Fi = np.fft.ifft(I, axis=0, norm='ortho')     # Fi[h,u]
    CHr = F.real.T.copy()                          # [h,u]
    CHi = F.imag.T.copy()
    EWr = F.real.T[:, :NV].copy()                  # [w,v]
    EWi = F.imag.T[:, :NV].copy()
    CHIr = Fi.real.T.copy()                        # [u,h]
    CHIi = Fi.imag.T.copy()
    Ir = np.eye(NV)
    Gc = np.fft.irfft(Ir, n=W, axis=-1, norm='ortho')        # [v,w]
    Gs = np.fft.irfft(1j * Ir, n=W, axis=-1, norm='ortho')   # [v,w]

    c = {}
    # CHpack [2][128, 258]: rows h (chunk), cols [CHr u=0..128 | CHi u=0..128]
    # (u=129..255 reconstructed from conjugate symmetry of the real-input FFT)
    c['chpack'] = np.stack([
        np.concatenate([CHr[j * 128:(j + 1) * 128, 0:129], CHi[j * 128:(j + 1) * 128, 0:129]], axis=1)
        for j in range(2)])
    # R1 [2][128, 258] = [EWr | EWi]; R2 = [-EWi | EWr] rows w chunk
    c['r1'] = np.stack([
        np.concatenate([EWr[j * 128:(j + 1) * 128], EWi[j * 128:(j + 1) * 128]], axis=1)
        for j in range(2)])
    c['r2'] = np.stack([
        np.concatenate([-EWi[j * 128:(j + 1) * 128], EWr[j * 128:(j + 1) * 128]], axis=1)
        for j in range(2)])
    c['r2n'] = -c['r2']
    # The S2 conj-symmetry trick stores the uc=1 block of Z in DESCENDING u
    # order (partition p holds u = 255-p). The whole pipeline is pointwise in
    # u until phase C's u-contraction, so only the chunk-1 inverse-DFT
    # constants need their rows reversed to match.
    # CHIpack [2][128, 512]: rows u chunk, cols [CHIr-h | CHIi-h]
    c['chipack'] = np.stack([
        np.concatenate([CHIr[0:128], CHIi[0:128]], axis=1),
        np.concatenate([CHIr[255:127:-1], CHIi[255:127:-1]], axis=1)])
    # NCHI [2][128, 256] = -CHIi rows u chunk
    c['nchi'] = np.stack([-CHIi[0:128], -CHIi[255:127:-1]])
    # NCHIpack [2][128, 512] = [-CHIi-h | CHIr-h] (merged Qr/Qi imag-term rhs)
    c['nchipack'] = np.stack([
        np.concatenate([-CHIi[0:128], CHIr[0:128]], axis=1),
        np.concatenate([-CHIi[255:127:-1], CHIr[255:127:-1]], axis=1)])
    # G tiles rows v=1..128
    c['gc'] = Gc[1:129]
    c['gs'] = Gs[1:129]
    return {k: v.astype(BF16) for k, v in c.items()}


def make_weight_consts(w1k, w2k):
    """Weight matrices for one block. w1k/w2k: (96, 96, 2) [i, o, ri]."""
    return {
        'w1r': w1k[..., 0].astype(BF16),
        'w1i': w1k[..., 1].astype(BF16),
        'w1in': (-w1k[..., 1]).astype(BF16),
        'w2r': w2k[..., 0].astype(BF16),
        'w2i': w2k[..., 1].astype(BF16),
        'w2in': (-w2k[..., 1]).astype(BF16),
    }


def build_nc():
    import concourse.bass as bass
    import concourse.tile as tile
    from concourse import bacc, mybir

    dt = mybir.dt
    nc = bacc.Bacc("TRN2", target_bir_lowering=False, debug=False)

    # I/O (x pre-split: h = hc*128 + p)
    xbf = nc.dram_tensor("xbf", [B, BLK, 2, 128, 256], dt.bfloat16, kind="ExternalInput").ap()
    chpack = nc.dram_tensor("chpack", [2, 128, 258], dt.bfloat16, kind="ExternalInput").ap()
    r1 = nc.dram_tensor("r1", [2, 128, 258], dt.bfloat16, kind="ExternalInput").ap()
    r2 = nc.dram_tensor("r2", [2, 128, 258], dt.bfloat16, kind="ExternalInput").ap()
    r2n = nc.dram_tensor("r2n", [2, 128, 258], dt.bfloat16, kind="ExternalInput").ap()
    chipack = nc.dram_tensor("chipack", [2, 128, 512], dt.bfloat16, kind="ExternalInput").ap()
    nchi = nc.dram_tensor("nchi", [2, 128, 256], dt.bfloat16, kind="ExternalInput").ap()
    nchipack = nc.dram_tensor("nchipack", [2, 128, 512], dt.bfloat16, kind="ExternalInput").ap()
    gc = nc.dram_tensor("gc", [128, 256], dt.bfloat16, kind="ExternalInput").ap()
    gs = nc.dram_tensor("gs", [128, 256], dt.bfloat16, kind="ExternalInput").ap()
    wts = {n: nc.dram_tensor(n, [96, 96], dt.bfloat16, kind="ExternalInput").ap()
           for n in ['w1r', 'w1i', 'w1in', 'w2r', 'w2i', 'w2in']}
    b1cols = nc.dram_tensor("b1cols", [96, 2], dt.float32, kind="ExternalInput").ap()
    # b2 clamp bounds broadcast across partitions: [128, 4, 96]
    # cols j: 0 = lam-b2r, 1 = -lam-b2r, 2 = lam-b2i, 3 = -lam-b2i (per channel)
    b2bc = nc.dram_tensor("b2bc", [128, 4, 96], dt.float32, kind="ExternalInput").ap()
    out = nc.dram_tensor("out", [B, BLK, 2, 128, 256], dt.float32, kind="ExternalOutput").ap()

    # DRAM scratch.
    # zbuf: Z spectrum, layout [b, p, uc, c, v2] with u = uc*128 + p
    zbuf = nc.dram_tensor("zbuf", [B, 128, 2, BLK, 258], dt.bfloat16).ap()
    # sbuf_d: S spectrum, layout [b, c, uc, p, v2]
    sbuf_d = nc.dram_tensor("sbufd", [B, BLK, 2, 128, 258], dt.bfloat16).ap()

    with tile.TileContext(nc) as tc:
        from contextlib import ExitStack
        with ExitStack() as ctx:
            consts = ctx.enter_context(tc.tile_pool(name="consts", bufs=1))
            pa_x = ctx.enter_context(tc.tile_pool(name="pa_x", bufs=8))
            pa_y = ctx.enter_context(tc.tile_pool(name="pa_y", bufs=8))
            pa_z = ctx.enter_context(tc.tile_pool(name="pa_z", bufs=8))
            pb_z = ctx.enter_context(tc.tile_pool(name="pb_z", bufs=8))
            pb_s = ctx.enter_context(tc.tile_pool(name="pb_s", bufs=8))
            pc_in = ctx.enter_context(tc.tile_pool(name="pc_in", bufs=8))
            pc_q = ctx.enter_context(tc.tile_pool(name="pc_q", bufs=8))
            pc_o = ctx.enter_context(tc.tile_pool(name="pc_o", bufs=8))
            # PSUM: 4 tags x 2 bufs x 1 bank = 8 banks
            psum = ctx.enter_context(tc.tile_pool(name="psum", bufs=1, space="PSUM"))

            # ---- Load constants ----
            def chunked_const(name, ap_, ncols):
                ts = []
                for j in range(2):
                    t = consts.tile([128, ncols], dt.bfloat16, tag=f"{name}{j}", name=f"{name}{j}")
                    nc.sync.dma_start(out=t, in_=ap_[j])
                    ts.append(t)
                return ts

            t_ch = chunked_const("t_ch", chpack, 258)
            t_r1 = chunked_const("t_r1", r1, 258)
            t_r2 = chunked_const("t_r2", r2, 258)
            t_r2n = chunked_const("t_r2n", r2n, 258)
            t_chi = chunked_const("t_chi", chipack, 512)
            t_nchi = chunked_const("t_nchi", nchi, 256)
            t_nchip = chunked_const("t_nchip", nchipack, 512)
            t_gc = consts.tile([128, 256], dt.bfloat16, tag="t_gc", name="t_gc")
            nc.sync.dma_start(out=t_gc, in_=gc)
            t_gs = consts.tile([128, 256], dt.bfloat16, tag="t_gs", name="t_gs")
            nc.sync.dma_start(out=t_gs, in_=gs)
            t_w = {}
            for n, ap_ in wts.items():
                t_w[n] = consts.tile([96, 96], dt.bfloat16, tag=f"t_{n}", name=f"t_{n}")
                nc.sync.dma_start(out=t_w[n], in_=ap_)

            t_b2bc = consts.tile([128, 4, 96], dt.float32, tag="t_b2bc", name="t_b2bc")
            nc.sync.dma_start(out=t_b2bc, in_=b2bc)
            t_b1 = consts.tile([96, 2], dt.float32, tag="t_b1", name="t_b1")
            nc.sync.dma_start(out=t_b1, in_=b1cols)

            for b in range(B):
                # ================= Phase A =================
                # x[h,w] --(DFT_h)--> Y[w, u] --(DFT_w)--> Z[u, v]
                for c in range(BLK):
                    xt = pa_x.tile([128, 2, 256], dt.bfloat16, tag="xt", name="xt")
                    nc.gpsimd.dma_start(out=xt, in_=xbf[b, c].transpose([1, 0, 2]))

                    # S1 with conj symmetry: Y[u] computed for u=0..128 only
                    # (N=258 matmuls); u=129..255 = conj(Y[256-u]) via the
                    # shifted uc=1 slices in S2 + host-reversed C constants.
                    ys = []
                    for wc in range(2):
                        psy = psum.tile([128, 258], dt.float32, tag="psA", name="psy", bufs=3)
                        nc.tensor.matmul(psy, lhsT=xt[:, 0, wc * 128:(wc + 1) * 128],
                                         rhs=t_ch[0], start=True, stop=False)
                        nc.tensor.matmul(psy, lhsT=xt[:, 1, wc * 128:(wc + 1) * 128],
                                         rhs=t_ch[1], start=False, stop=True)
                        # y layout: [Yr u=0..128 | Yi u=0..128]
                        y = pa_y.tile([128, 258], dt.bfloat16, tag=f"y{wc}", name=f"y{wc}")
                        if wc == 0:
                            nc.scalar.copy(y, psy)
                        else:
                            nc.vector.tensor_scalar_add(y, psy, 0.0)
                        ys.append(y)

                    zt = pa_z.tile([128, 2, 258], dt.bfloat16, tag="zt", name="zt")
                    for uc in range(2):
                        psz = psum.tile([128, 258], dt.float32, tag="psB", name="psz", bufs=2)
                        if uc == 0:
                            # u = 0..127: Y[u] directly
                            sre, sim, tr2 = slice(0, 128), slice(129, 257), t_r2
                        else:
                            # partition p holds u = 255-p via conj(Y[s]), s = p+1
                            sre, sim, tr2 = slice(1, 129), slice(130, 258), t_r2n
                        nc.tensor.matmul(psz, lhsT=ys[0][:, sre], rhs=t_r1[0], start=True, stop=False)
                        nc.tensor.matmul(psz, lhsT=ys[0][:, sim], rhs=tr2[0], start=False, stop=False)
                        nc.tensor.matmul(psz, lhsT=ys[1][:, sre], rhs=t_r1[1], start=False, stop=False)
                        nc.tensor.matmul(psz, lhsT=ys[1][:, sim], rhs=tr2[1], start=False, stop=True)
                        if uc == 0:
                            nc.scalar.copy(zt[:, uc, :], psz)
                        else:
                            nc.vector.tensor_scalar_add(zt[:, uc, :], psz, 0.0)
                    nc.sync.dma_start(out=zbuf[b, :, :, c, :], in_=zt)

                # ================= Phase B =================
                # per u: o2[c, v] = W2 relu(W1 Z + b1) raw mix2
                # (softshrink + b2 deferred to phase C via clamp bounds)
                # processed as 8-u DMA groups, pairs of u per matmul set
                for uc in range(2):
                    for p0 in range(0, 128, 8):
                        zr8 = pb_z.tile([96, 8, 129], dt.bfloat16, tag="zr8", name="zr8")
                        nc.sync.dma_start(
                            out=zr8, in_=zbuf[b, p0:p0 + 8, uc, :, 0:129].transpose([1, 0, 2]))
                        zi8 = pb_z.tile([96, 8, 129], dt.bfloat16, tag="zi8", name="zi8")
                        nc.sync.dma_start(
                            out=zi8, in_=zbuf[b, p0:p0 + 8, uc, :, 129:258].transpose([1, 0, 2]))
                        st8 = pb_s.tile([96, 8, 258], dt.bfloat16, tag="st8", name="st8")

                        for j in range(4):
                            js = slice(2 * j, 2 * j + 2)
                            zrs = zr8[:, js, :]
                            zis = zi8[:, js, :]
                            # mix1: o1r = W1r Zr - W1i Zi ; o1i = W1i Zr + W1r Zi
                            ps1r = psum.tile([96, 2, 129], dt.float32, tag="psA", name="ps1r", bufs=3)
                            ps1i = psum.tile([96, 2, 129], dt.float32, tag="psA", name="ps1i", bufs=3)
                            nc.tensor.matmul(ps1r, lhsT=t_w['w1r'], rhs=zrs, start=True, stop=False)
                            nc.tensor.matmul(ps1i, lhsT=t_w['w1r'], rhs=zis, start=True, stop=False)
                            nc.tensor.matmul(ps1r, lhsT=t_w['w1in'], rhs=zis, start=False, stop=True)
                            nc.tensor.matmul(ps1i, lhsT=t_w['w1i'], rhs=zrs, start=False, stop=True)

                            o1r = pb_s.tile([96, 2, 129], dt.bfloat16, tag="o1r", name="o1r")
                            nc.scalar.activation(o1r, ps1r, mybir.ActivationFunctionType.Relu,
                                                 bias=t_b1[:, 0:1])
                            o1i = pb_s.tile([96, 2, 129], dt.bfloat16, tag="o1i", name="o1i")
                            nc.scalar.activation(o1i, ps1i, mybir.ActivationFunctionType.Relu,
                                                 bias=t_b1[:, 1:2])

                            ps2r = psum.tile([96, 2, 129], dt.float32, tag="psB", name="ps2r", bufs=2)
                            ps2i = psum.tile([96, 2, 129], dt.float32, tag="psC", name="ps2i", bufs=2)
                            nc.tensor.matmul(ps2r, lhsT=t_w['w2r'], rhs=o1r, start=True, stop=False)
                            nc.tensor.matmul(ps2i, lhsT=t_w['w2r'], rhs=o1i, start=True, stop=False)
                            nc.tensor.matmul(ps2r, lhsT=t_w['w2in'], rhs=o1i, start=False, stop=True)
                            nc.tensor.matmul(ps2i, lhsT=t_w['w2i'], rhs=o1r, start=False, stop=True)

                            nc.vector.tensor_scalar_add(st8[:, js, 0:129], ps2r, 0.0)
                            nc.vector.tensor_scalar_add(st8[:, js, 129:258], ps2i, 0.0)

                        nc.sync.dma_start(out=sbuf_d[b, :, uc, p0:p0 + 8, :], in_=st8)

                # ================= Phase C =================
                # T = softshrink_b2(o2) + Z; out[h, w] = irfft2(T)
                for c in range(BLK):
                    o2t = pc_in.tile([128, 2, 258], dt.bfloat16, tag="o2t", name="o2t")
                    nc.gpsimd.dma_start(out=o2t, in_=sbuf_d[b, c].transpose([1, 0, 2]))
                    ztc = pc_in.tile([128, 2, 258], dt.bfloat16, tag="ztc", name="ztc")
                    nc.sync.dma_start(out=ztc, in_=zbuf[b, :, :, c, :])
                    # cl = clamp(o2, -lam-b2, lam-b2) per re/im half
                    cl2 = pc_in.tile([128, 2, 258], dt.bfloat16, tag="cl2", name="cl2")
                    nc.vector.tensor_scalar(cl2[:, :, 0:129], o2t[:, :, 0:129],
                                            t_b2bc[:, 0, c:c + 1], t_b2bc[:, 1, c:c + 1],
                                            mybir.AluOpType.min, mybir.AluOpType.max)
                    nc.gpsimd.tensor_scalar(cl2[:, :, 129:258], o2t[:, :, 129:258],
                                            t_b2bc[:, 2, c:c + 1], t_b2bc[:, 3, c:c + 1],
                                            mybir.AluOpType.min, mybir.AluOpType.max)
                    # T = (o2 - cl) + Z
                    tt1 = pc_in.tile([128, 2, 258], dt.bfloat16, tag="tt1", name="tt1")
                    nc.vector.tensor_tensor(tt1, o2t, cl2, mybir.AluOpType.subtract)
                    tt = pc_in.tile([128, 2, 258], dt.bfloat16, tag="tt", name="tt")
                    nc.vector.tensor_tensor(tt, tt1, ztc, mybir.AluOpType.add)

                    # [QrT | QiT] in one psum bank: Qr = tr.CHIr - ti.CHIi ;
                    # Qi = tr.CHIi + ti.CHIr (chi = [CHIr|CHIi], nchip = [-CHIi|CHIr])
                    # The 8 tiny DC (v=0) matmuls are interleaved between the big
                    # ones so their full-width LDWEIGHTS hide behind long streams.
                    psab = psum.tile([128, 512], dt.float32, tag="psA", name="psab", bufs=3)
                    psq = psum.tile([128, 2], dt.float32, tag="psD", name="psq", bufs=1)

                    def dc_mm(k):
                        hc, t = divmod(k, 4)
                        lhs = [t_chi[0], t_nchi[0], t_chi[1], t_nchi[1]][t]
                        rhs = [tt[:, 0, 0:1], tt[:, 0, 129:130],
                               tt[:, 1, 0:1], tt[:, 1, 129:130]][t]
                        hs = slice(hc * 128, (hc + 1) * 128)
                        nc.tensor.matmul(psq[:, hc:hc + 1], lhsT=lhs[:, hs], rhs=rhs,
                                         start=(k == 0), stop=(k == 7), skip_group_check=True)

                    nc.tensor.matmul(psab, lhsT=tt[:, 0, 1:129], rhs=t_chi[0], start=True, stop=False)
                    dc_mm(0)
                    nc.tensor.matmul(psab, lhsT=tt[:, 1, 1:129], rhs=t_chi[1], start=False, stop=False)
                    dc_mm(1)
                    nc.tensor.matmul(psab, lhsT=tt[:, 0, 130:258], rhs=t_nchip[0], start=False, stop=False)
                    dc_mm(2)
                    nc.tensor.matmul(psab, lhsT=tt[:, 1, 130:258], rhs=t_nchip[1], start=False, stop=True)
                    dc_mm(3)

                    qr = pc_q.tile([128, 256], dt.bfloat16, tag="qr", name="qr")
                    nc.scalar.copy(qr, psab[:, 0:256])
                    qi = pc_q.tile([128, 256], dt.bfloat16, tag="qi", name="qi")
                    nc.scalar.copy(qi, psab[:, 256:512])

                    ot = pc_o.tile([128, 2, 256], dt.float32, tag="ot", name="ot")
                    psos = []
                    for hc in range(2):
                        pso = psum.tile([128, 256], dt.float32, tag="psC", name="pso", bufs=2)
                        nc.tensor.matmul(pso, lhsT=qr[:, hc * 128:(hc + 1) * 128], rhs=t_gc,
                                         start=True, stop=False)
                        dc_mm(4 + 2 * hc)
                        nc.tensor.matmul(pso, lhsT=qi[:, hc * 128:(hc + 1) * 128], rhs=t_gs,
                                         start=False, stop=True)
                        dc_mm(5 + 2 * hc)
                        psos.append(pso)
                    q0 = pc_q.tile([128, 2], dt.float32, tag="q0", name="q0")
                    nc.vector.tensor_scalar_mul(q0, psq, 1.0 / 16.0)
                    nc.scalar.activation(ot[:, 0, :], psos[0],
                                         mybir.ActivationFunctionType.Identity,
                                         bias=q0[:, 0:1])
                    nc.vector.tensor_scalar_add(ot[:, 1, :], psos[1], q0[:, 1:2])
                    nc.sync.dma_start(out=out[b, c].transpose([1, 0, 2]), in_=ot)
    nc.compile()
    return nc


_NC_CACHE = {}


def _get_nc():
    if 'nc' not in _NC_CACHE:
        _NC_CACHE['nc'] = build_nc()
    return _NC_CACHE['nc']


def make_in_maps(x, w1, b1, w2, b2):
    hc = make_host_consts()
    x = np.ascontiguousarray(x, dtype=np.float32)
    in_maps = []
    for k in range(NCORES):
        xk = np.ascontiguousarray(x[:, BLK * k:BLK * (k + 1)])
        wk = make_weight_consts(w1[k], w2[k])
        b2k = b2[k, :, 0, 0, :]
        b2cols = np.stack([LAM - b2k[:, 0], -LAM - b2k[:, 0],
                           LAM - b2k[:, 1], -LAM - b2k[:, 1]], axis=1).astype(np.float32)
        b2bc = np.ascontiguousarray(np.broadcast_to(b2cols.T[None, :, :], (128, 4, 96)))
        b1cols = np.ascontiguousarray(b1[k, :, 0, 0, :], dtype=np.float32)
        m = dict(
            b1cols=b1cols,
            b2bc=b2bc,
            xbf=xk.astype(BF16).reshape(B, BLK, 2, 128, 256),
            chpack=hc['chpack'], r1=hc['r1'], r2=hc['r2'], r2n=hc['r2n'],
            chipack=hc['chipack'], nchi=hc['nchi'], nchipack=hc['nchipack'],
            gc=hc['gc'], gs=hc['gs'],
            **wk,
        )
        in_maps.append(m)
    return in_maps


def assemble(res):
    outs = [res.results[k]['out'].reshape(B, BLK, H, W) for k in range(NCORES)]
    return np.concatenate(outs, axis=1)


def kernel(x, w1, b1, w2, b2):
    from concourse.bass_utils import run_bass_kernel_spmd
    nc = _get_nc()
    in_maps = make_in_maps(np.asarray(x), np.asarray(w1), np.asarray(b1),
                           np.asarray(w2), np.asarray(b2))
    res = run_bass_kernel_spmd(nc, in_maps, core_ids=list(range(NCORES)))
    return assemble(res)
